# revision 6
# baseline (speedup 1.0000x reference)
"""Multi-head attention (B=4, S=2048, E=1024, H=16) on 8 Trainium2 NeuronCores.

Sharding: 2 cores per batch element (data-parallel over B=4), each core
computes 8 of the 16 heads (tensor-parallel over H). Each core:
  - qkT = (W_qk_local)^T @ x^T            [1024f, 2048s]  (f = q|k heads)
  - v   = x @ W_v_local (+bias via ones)  [2048s, 8*65]   (65th col = ones)
  - per head: scoresT = kT^T q (j on partitions), exp on ACT (no max
    subtraction needed -- scores are small), AV via v_aug^T @ expT which
    also yields the softmax denominators in the extra row, transposed
    normalized attention written via PE transposes, ctx normalized with a
    broadcast reciprocal, projection partial out = ctxT^T @ W_proj_local.
Host sums the two per-batch projection partials and adds b_proj.
"""

import os
import sys
import types

for _p in ("/opt/trn_rl_repo", "/root/.axon_site/_ro/trn_rl_repo"):
    if os.path.isdir(_p) and _p not in sys.path:
        sys.path.append(_p)

import numpy as np
import ml_dtypes

import concourse.bass as bass
import concourse.bacc as bacc
import concourse.tile as tile
from concourse import mybir
from concourse.bass_utils import run_bass_kernel_spmd
from concourse.masks import make_identity
from concourse.vector_clock import ScopedClock, VectorClock

BF16 = mybir.dt.bfloat16
F32 = mybir.dt.float32
NP_BF16 = ml_dtypes.bfloat16

B, S, E, H, D = 4, 2048, 1024, 16, 64
NCORES = 8
NH = 8          # local heads per core
HD = NH * D     # 512 local head-dim columns per of q/k/v
SCALE = 1.0 / np.sqrt(D)


def _install_ntff_hook():
    """The agent image's antenv lacks axon_hooks; inject it so trace=True works."""
    try:
        import antenv.axon_hooks  # noqa: F401
        return
    except ImportError:
        pass
    try:
        import antenv
        from trn_agent_boot.trn_boot import _ntff_profile_via_ctypes
    except ImportError:
        return
    mod = types.ModuleType("antenv.axon_hooks")
    mod._hook = None

    def set_axon_ntff_profile_hook(h):
        mod._hook = h

    def get_axon_ntff_profile_hook():
        return mod._hook

    mod.set_axon_ntff_profile_hook = set_axon_ntff_profile_hook
    mod.get_axon_ntff_profile_hook = get_axon_ntff_profile_hook
    sys.modules["antenv.axon_hooks"] = mod
    antenv.axon_hooks = mod
    try:
        mod._hook = _ntff_profile_via_ctypes("/opt/axon/libaxon_pjrt.so")
    except Exception:
        mod._hook = None


class ChunkedDrainTileContext(tile.TileContext):
    """TileContext whose exit drain splits semaphore waits across multiple
    drain instructions (this walrus encodes only one sync wait per CTRL)."""

    def _drain_and_barrier(self, tick_clock, wait_clock):
        gc = tick_clock.global_clock
        n = len(gc)
        ticks = [(p, gc[p]) for p in range(n) if gc[p] > 0]
        if not ticks:
            self.nc.sync.drain()
        for p, t in ticks:
            vc = VectorClock([0] * n)
            vc.require_at_least(p, t)
            d = self.nc.sync.drain()
            wait_clock.add_sem_waits(d.ins, ScopedClock({None: vc}))
        self.nc.all_engine_barrier()
        assert self.sems is not None
        popped = self.nc._tile_sem_poison_stack.pop()
        assert popped is self._sem_poison
        self.nc.clear_and_free_semaphores(list(self.sems.allocated().values()))
        self.nc.all_engine_barrier()


def _build_kernel():
    nc = bacc.Bacc("TRN2")

    xT = nc.dram_tensor("xT", [E, S], BF16, kind="ExternalInput")
    wqk = nc.dram_tensor("wqk", [E, 2 * HD], BF16, kind="ExternalInput")
    wv = nc.dram_tensor("wv", [E, HD], BF16, kind="ExternalInput")
    bqk = nc.dram_tensor("bqk", [2 * HD], F32, kind="ExternalInput")
    bv = nc.dram_tensor("bv", [HD], BF16, kind="ExternalInput")
    wproj = nc.dram_tensor("wproj", [HD, E], BF16, kind="ExternalInput")
    attn_o = nc.dram_tensor("attn_o", [NH, S, S], BF16, kind="ExternalOutput")
    out_o = nc.dram_tensor("out_o", [S, E], F32, kind="ExternalOutput")

    ET = E // 128   # 8 e-tiles (contraction tiles for qkv matmuls)
    ST = S // 128   # 16 s-tiles
    JT = S // 128   # 16 j-tiles per head
    IH = 2          # i halves
    IW = S // IH    # 1024 i per half
    ICN = IW // 512  # 2 512-wide i chunks per half

    with ChunkedDrainTileContext(nc) as tc:
        with tc.tile_pool(name="persist", bufs=1) as persist:
            ident = persist.tile([128, 128], BF16)
            make_identity(nc, ident)
            idf = persist.tile([1, 1], F32)
            nc.vector.memset(idf, 1.0)
            ones = persist.tile([1, 128], BF16)
            nc.vector.memset(ones, 1.0)
            bqk_sb = persist.tile([128, 2 * HD // 128], F32)
            nc.gpsimd.dma_start(out=bqk_sb, in_=bqk.rearrange("(t p) -> p t", p=128))
            bv_sb = persist.tile([1, HD], BF16)
            nc.gpsimd.dma_start(out=bv_sb, in_=bv[None, :])

            # persistent activations
            qk_sb = [persist.tile([128, S], BF16, tag=f"qk{i}", name=f"qk{i}")
                     for i in range(2 * HD // 128)]
            v_sb = [persist.tile([128, NH * (D + 1)], BF16, tag=f"v{i}", name=f"v{i}")
                    for i in range(ST)]
            wproj_sb = [persist.tile([128, E], BF16, tag=f"wp{i}", name=f"wp{i}")
                        for i in range(HD // 128)]
            ctx_sb = [persist.tile([128, S], BF16, tag=f"ctx{i}", name=f"ctx{i}")
                      for i in range(NH // 2)]

            for t in range(HD // 128):
                nc.sync.dma_start(out=wproj_sb[t], in_=wproj[t * 128:(t + 1) * 128, :])

            # ---- phase 1: qkT and v ----
            with tc.tile_pool(name="ph1", bufs=1) as ph1:
                xT_sb = [ph1.tile([128, S], BF16, tag=f"xT{i}", name=f"xT{i}") for i in range(ET)]
                wqk_sb = [ph1.tile([128, 2 * HD], BF16, tag=f"wqk{i}", name=f"wqk{i}")
                          for i in range(ET)]
                wv_sb = [ph1.tile([128, HD], BF16, tag=f"wv{i}", name=f"wv{i}") for i in range(ET)]
                for t in range(ET):
                    nc.sync.dma_start(out=xT_sb[t], in_=xT[t * 128:(t + 1) * 128, :])
                    nc.sync.dma_start(out=wqk_sb[t], in_=wqk[t * 128:(t + 1) * 128, :])
                    nc.sync.dma_start(out=wv_sb[t], in_=wv[t * 128:(t + 1) * 128, :])

                with tc.tile_pool(name="ph1ps", bufs=4, space="PSUM") as pp:
                    for ft in range(2 * HD // 128):
                        for sc in range(S // 512):
                            ps = pp.tile([128, 512], F32, tag="p1")
                            for et in range(ET):
                                nc.tensor.matmul(
                                    ps,
                                    wqk_sb[et][:, ft * 128:(ft + 1) * 128],
                                    xT_sb[et][:, sc * 512:(sc + 1) * 512],
                                    start=(et == 0),
                                    stop=(et == ET - 1),
                                )
                            nc.scalar.activation(
                                qk_sb[ft][:, sc * 512:(sc + 1) * 512],
                                ps,
                                mybir.ActivationFunctionType.Identity,
                                bias=bqk_sb[:, ft:ft + 1],
                            )
                    for st in range(ST):
                        nc.vector.memset(v_sb[st], 1.0)
                        ps = pp.tile([128, 512], F32, tag="p1")
                        for et in range(ET):
                            nc.tensor.matmul(
                                ps,
                                xT_sb[et][:, st * 128:(st + 1) * 128],
                                wv_sb[et],
                                start=(et == 0),
                                stop=False,
                            )
                        nc.tensor.matmul(
                            ps, ones, bv_sb, start=False, stop=True,
                        )
                        nc.scalar.activation(
                            v_sb[st].rearrange("p (h d) -> p h d", d=D + 1)[:, :, 0:D],
                            ps.rearrange("p (h d) -> p h d", d=D),
                            mybir.ActivationFunctionType.Copy,
                        )

            # ---- phase 2: attention per head ----
            with (
                tc.tile_pool(name="expp", bufs=2 * JT) as expp,
                tc.tile_pool(name="stagep", bufs=4) as stagep,
                tc.tile_pool(name="outp", bufs=2) as outp,
                tc.tile_pool(name="smallp", bufs=2) as smallp,
                tc.tile_pool(name="dramp", bufs=2, space="DRAM") as dramp,
                tc.tile_pool(name="psA", bufs=3, space="PSUM") as psA,
                tc.tile_pool(name="psC", bufs=2, space="PSUM") as psC,
            ):
                for h in range(NH):
                    qt = qk_sb[h // 2]
                    kt = qk_sb[NH // 2 + h // 2]
                    po = (h % 2) * D   # partition offset within the f-tile
                    for ih in range(IH):
                        i0 = ih * IW
                        ctx_ps = psC.tile([D + 1, IW], F32, tag="ctx")
                        exp_tiles = []
                        for jt in range(JT):
                            etile = expp.tile([128, IW], BF16, tag="exp")
                            for ic in range(ICN):
                                ps = psA.tile([128, 512], F32, tag="ps")
                                nc.tensor.matmul(
                                    ps,
                                    kt[po:po + D, jt * 128:(jt + 1) * 128],
                                    qt[po:po + D, i0 + ic * 512: i0 + (ic + 1) * 512],
                                    start=True,
                                    stop=True,
                                )
                                nc.scalar.activation(
                                    etile[:, ic * 512:(ic + 1) * 512],
                                    ps,
                                    mybir.ActivationFunctionType.Exp,
                                    scale=float(SCALE),
                                )
                                nc.tensor.matmul(
                                    ctx_ps[:, ic * 512:(ic + 1) * 512],
                                    v_sb[jt][:, h * (D + 1): h * (D + 1) + D + 1],
                                    etile[:, ic * 512:(ic + 1) * 512],
                                    start=(jt == 0),
                                    stop=(jt == JT - 1),
                                    skip_group_check=True,
                                )
                            exp_tiles.append(etile)

                        # softmax denominators -> reciprocals (row + transposed)
                        recip_row = smallp.tile([1, IW], F32, tag="recip")
                        nc.vector.reciprocal(recip_row, ctx_ps[D:D + 1, :])
                        rt_ps = psA.tile([128, 512], F32, tag="ps")
                        for c in range(IW // 128):
                            nc.tensor.transpose(
                                rt_ps[:, c:c + 1],
                                recip_row[:, c * 128:(c + 1) * 128],
                                idf,
                            )
                        rt_sb = smallp.tile([128, IW // 128], F32, tag="rt")
                        nc.vector.tensor_copy(rt_sb, rt_ps[:, 0:IW // 128])

                        # normalize ctx: broadcast recip along partitions via a
                        # DRAM bounce (SBUF APs reject stride-0 partitions)
                        rscratch = dramp.tile([1, IW], F32, tag="rscratch")
                        nc.gpsimd.dma_start(out=rscratch, in_=recip_row)
                        rb = smallp.tile([D, IW], F32, tag="rb")
                        bcast_ap = bass.AP(
                            tensor=rscratch.tensor,
                            offset=rscratch.offset,
                            ap=[[0, D]] + [list(p) for p in rscratch.ap[1:]],
                        )
                        nc.gpsimd.dma_start(out=rb, in_=bcast_ap)
                        nc.vector.tensor_mul(
                            ctx_sb[h // 2][po:po + D, i0:i0 + IW],
                            ctx_ps[0:D, :],
                            rb,
                        )

                        # transposed normalized attention -> HBM
                        for it in range(IW // 128):
                            st_tile = stagep.tile([128, S], BF16, tag="stage")
                            for jq in range(JT // 4):
                                tp = psA.tile([128, 512], BF16, tag="ps")
                                for j4 in range(4):
                                    jt = jq * 4 + j4
                                    nc.tensor.transpose(
                                        tp[:, j4 * 128:(j4 + 1) * 128],
                                        exp_tiles[jt][:, it * 128:(it + 1) * 128],
                                        ident,
                                    )
                                dst = st_tile[:, jq * 512:(jq + 1) * 512]
                                if jq % 2 == 0:
                                    nc.vector.tensor_scalar_mul(
                                        dst, tp, rt_sb[:, it:it + 1]
                                    )
                                else:
                                    nc.scalar.activation(
                                        dst,
                                        tp,
                                        mybir.ActivationFunctionType.Copy,
                                        scale=rt_sb[:, it:it + 1],
                                    )
                            nc.sync.dma_start(
                                out=attn_o[h, i0 + it * 128: i0 + (it + 1) * 128, :],
                                in_=st_tile,
                            )

                # ---- phase 3: projection partial ----
                for sti in range(ST):
                    osb = outp.tile([128, E], F32, tag="out")
                    for ec in range(E // 512):
                        ps = psA.tile([128, 512], F32, tag="ps")
                        for hp in range(NH // 2):
                            nc.tensor.matmul(
                                ps,
                                ctx_sb[hp][:, sti * 128:(sti + 1) * 128],
                                wproj_sb[hp][:, ec * 512:(ec + 1) * 512],
                                start=(hp == 0),
                                stop=(hp == NH // 2 - 1),
                            )
                        nc.scalar.copy(osb[:, ec * 512:(ec + 1) * 512], ps)
                    nc.sync.dma_start(out=out_o[sti * 128:(sti + 1) * 128, :], in_=osb)

    nc.finalize()
    return nc


_NC = None
LAST_EXEC_NS = None
LAST_RESULTS = None


def kernel(x, W_qkv, b_qkv, W_proj, b_proj):
    global _NC, LAST_EXEC_NS, LAST_RESULTS
    x = np.asarray(x, dtype=np.float32)
    W_qkv = np.asarray(W_qkv, dtype=np.float32)
    b_qkv = np.asarray(b_qkv, dtype=np.float32)
    W_proj = np.asarray(W_proj, dtype=np.float32)
    b_proj = np.asarray(b_proj, dtype=np.float32)

    if _NC is None:
        _NC = _build_kernel()
    nc = _NC

    # per-head column blocks of W_qkv: q at h*D, k at E + h*D, v at 2E + h*D
    Wq = W_qkv[:, :E].reshape(E, H, D)
    Wk = W_qkv[:, E:2 * E].reshape(E, H, D)
    Wv = W_qkv[:, 2 * E:].reshape(E, H, D)
    bq = b_qkv[:E].reshape(H, D)
    bk = b_qkv[E:2 * E].reshape(H, D)
    bvv = b_qkv[2 * E:].reshape(H, D)

    in_maps = []
    for c in range(NCORES):
        b = c // 2
        hh = c % 2
        hs = slice(hh * NH, (hh + 1) * NH)
        wqk_l = np.concatenate(
            [Wq[:, hs].reshape(E, HD), Wk[:, hs].reshape(E, HD)], axis=1
        )
        bqk_l = np.concatenate([bq[hs].reshape(HD), bk[hs].reshape(HD)])
        wv_l = Wv[:, hs].reshape(E, HD)
        bv_l = bvv[hs].reshape(HD)
        wproj_l = W_proj[hh * HD:(hh + 1) * HD, :]
        in_maps.append({
            "xT": np.ascontiguousarray(x[b].T).astype(NP_BF16),
            "wqk": wqk_l.astype(NP_BF16),
            "wv": wv_l.astype(NP_BF16),
            "bqk": bqk_l.astype(np.float32),
            "bv": bv_l.astype(NP_BF16),
            "wproj": wproj_l.astype(NP_BF16),
        })

    trace = bool(os.environ.get("TRNKERNEL_TRACE"))
    kwargs = {}
    if trace:
        _install_ntff_hook()
        kwargs["trace"] = True
        tdir = os.environ.get("TRNKERNEL_TRACE_DIR")
        if tdir:
            os.makedirs(tdir, exist_ok=True)
            kwargs["tmpdir"] = tdir
    res = run_bass_kernel_spmd(nc, in_maps, core_ids=list(range(NCORES)), **kwargs)
    LAST_EXEC_NS = res.exec_time_ns
    LAST_RESULTS = res

    attn = np.empty((B, H, S, S), np.float32)
    out = np.empty((B, S, E), np.float32)
    for c in range(NCORES):
        b = c // 2
        hh = c % 2
        attn[b, hh * NH:(hh + 1) * NH] = res.results[c]["attn_o"].astype(np.float32)
    for b in range(B):
        out[b] = res.results[2 * b]["out_o"] + res.results[2 * b + 1]["out_o"] + b_proj
    return out, attn


# revision 8
# speedup vs baseline: 1.2061x; 1.2061x over previous
"""Multi-head attention (B=4, S=2048, E=1024, H=16) on 8 Trainium2 NeuronCores.

Sharding: 2 cores per batch element (data-parallel over B=4), each core
computes 8 of the 16 heads (tensor-parallel over H). Each core:
  - qkT = (W_qk_local)^T @ x^T            [1024f, 2048s]  (f = q|k heads)
  - v   = x @ W_v_local (+bias via ones)  [2048s, 8*65]   (65th col = ones)
  - per head: scoresT = kT^T q (j on partitions), exp on ACT (no max
    subtraction needed -- scores are small), AV via v_aug^T @ expT which
    also yields the softmax denominators in the extra row, transposed
    normalized attention written via PE transposes, ctx normalized with a
    broadcast reciprocal, projection partial out = ctxT^T @ W_proj_local.
Host sums the two per-batch projection partials and adds b_proj.
"""

import os
import sys
import types

for _p in ("/opt/trn_rl_repo", "/root/.axon_site/_ro/trn_rl_repo"):
    if os.path.isdir(_p) and _p not in sys.path:
        sys.path.append(_p)

import numpy as np
import ml_dtypes

import concourse.bass as bass
import concourse.bacc as bacc
import concourse.tile as tile
from concourse import mybir
from concourse.bass_utils import run_bass_kernel_spmd
from concourse.masks import make_identity
from concourse.vector_clock import ScopedClock, VectorClock

BF16 = mybir.dt.bfloat16
F32 = mybir.dt.float32
NP_BF16 = ml_dtypes.bfloat16

B, S, E, H, D = 4, 2048, 1024, 16, 64
NCORES = 8
NH = 8          # local heads per core
HD = NH * D     # 512 local head-dim columns per of q/k/v
SCALE = 1.0 / np.sqrt(D)


def _install_ntff_hook():
    """The agent image's antenv lacks axon_hooks; inject it so trace=True works."""
    try:
        import antenv.axon_hooks  # noqa: F401
        return
    except ImportError:
        pass
    try:
        import antenv
        from trn_agent_boot.trn_boot import _ntff_profile_via_ctypes
    except ImportError:
        return
    mod = types.ModuleType("antenv.axon_hooks")
    mod._hook = None

    def set_axon_ntff_profile_hook(h):
        mod._hook = h

    def get_axon_ntff_profile_hook():
        return mod._hook

    mod.set_axon_ntff_profile_hook = set_axon_ntff_profile_hook
    mod.get_axon_ntff_profile_hook = get_axon_ntff_profile_hook
    sys.modules["antenv.axon_hooks"] = mod
    antenv.axon_hooks = mod
    try:
        mod._hook = _ntff_profile_via_ctypes("/opt/axon/libaxon_pjrt.so")
    except Exception:
        mod._hook = None


class ChunkedDrainTileContext(tile.TileContext):
    """TileContext whose exit drain splits semaphore waits across multiple
    drain instructions (this walrus encodes only one sync wait per CTRL)."""

    def _drain_and_barrier(self, tick_clock, wait_clock):
        gc = tick_clock.global_clock
        n = len(gc)
        ticks = [(p, gc[p]) for p in range(n) if gc[p] > 0]
        if not ticks:
            self.nc.sync.drain()
        for p, t in ticks:
            vc = VectorClock([0] * n)
            vc.require_at_least(p, t)
            d = self.nc.sync.drain()
            wait_clock.add_sem_waits(d.ins, ScopedClock({None: vc}))
        self.nc.all_engine_barrier()
        assert self.sems is not None
        popped = self.nc._tile_sem_poison_stack.pop()
        assert popped is self._sem_poison
        self.nc.clear_and_free_semaphores(list(self.sems.allocated().values()))
        self.nc.all_engine_barrier()


def _build_kernel():
    nc = bacc.Bacc("TRN2")

    xT = nc.dram_tensor("xT", [E, S], BF16, kind="ExternalInput")
    wqk = nc.dram_tensor("wqk", [E, 2 * HD], BF16, kind="ExternalInput")
    wv = nc.dram_tensor("wv", [E, HD], BF16, kind="ExternalInput")
    bqk = nc.dram_tensor("bqk", [2 * HD], F32, kind="ExternalInput")
    bv = nc.dram_tensor("bv", [HD], BF16, kind="ExternalInput")
    wproj = nc.dram_tensor("wproj", [HD, E], BF16, kind="ExternalInput")
    attn_o = nc.dram_tensor("attn_o", [NH, S, S], BF16, kind="ExternalOutput")
    out_o = nc.dram_tensor("out_o", [S, E], F32, kind="ExternalOutput")

    ET = E // 128   # 8 e-tiles (contraction tiles for qkv matmuls)
    ST = S // 128   # 16 s-tiles
    JT = S // 128   # 16 j-tiles per head
    IH = 2          # i halves
    IW = S // IH    # 1024 i per half
    ICN = IW // 512  # 2 512-wide i chunks per half

    with ChunkedDrainTileContext(nc) as tc:
        with tc.tile_pool(name="persist", bufs=1) as persist:
            ident = persist.tile([128, 128], BF16)
            make_identity(nc, ident)
            idf = persist.tile([1, 1], F32)
            nc.vector.memset(idf, 1.0)
            ones = persist.tile([1, 128], BF16)
            nc.vector.memset(ones, 1.0)
            bqk_sb = persist.tile([128, 2 * HD // 128], F32)
            nc.gpsimd.dma_start(out=bqk_sb, in_=bqk.rearrange("(t p) -> p t", p=128))
            bv_sb = persist.tile([1, HD], BF16)
            nc.gpsimd.dma_start(out=bv_sb, in_=bv[None, :])

            # persistent activations
            qk_sb = [persist.tile([128, S], BF16, tag=f"qk{i}", name=f"qk{i}")
                     for i in range(2 * HD // 128)]
            v_sb = [persist.tile([128, NH * (D + 1)], BF16, tag=f"v{i}", name=f"v{i}")
                    for i in range(ST)]
            wproj_sb = [persist.tile([128, E], BF16, tag=f"wp{i}", name=f"wp{i}")
                        for i in range(HD // 128)]
            ctx_sb = [persist.tile([128, S], BF16, tag=f"ctx{i}", name=f"ctx{i}")
                      for i in range(NH // 2)]

            for t in range(HD // 128):
                nc.sync.dma_start(out=wproj_sb[t], in_=wproj[t * 128:(t + 1) * 128, :])

            # ---- phase 1: qkT and v ----
            with tc.tile_pool(name="ph1", bufs=1) as ph1:
                xT_sb = [ph1.tile([128, S], BF16, tag=f"xT{i}", name=f"xT{i}") for i in range(ET)]
                wqk_sb = [ph1.tile([128, 2 * HD], BF16, tag=f"wqk{i}", name=f"wqk{i}")
                          for i in range(ET)]
                wv_sb = [ph1.tile([128, HD], BF16, tag=f"wv{i}", name=f"wv{i}") for i in range(ET)]
                for t in range(ET):
                    nc.sync.dma_start(out=xT_sb[t], in_=xT[t * 128:(t + 1) * 128, :])
                    nc.sync.dma_start(out=wqk_sb[t], in_=wqk[t * 128:(t + 1) * 128, :])
                    nc.sync.dma_start(out=wv_sb[t], in_=wv[t * 128:(t + 1) * 128, :])

                with tc.tile_pool(name="ph1ps", bufs=2, space="PSUM") as pp:
                    for ft in range(2 * HD // 128):
                        ps = pp.tile([128, S], F32, tag="p1w")
                        for sc in range(S // 512):
                            for et in range(ET):
                                nc.tensor.matmul(
                                    ps[:, sc * 512:(sc + 1) * 512],
                                    wqk_sb[et][:, ft * 128:(ft + 1) * 128],
                                    xT_sb[et][:, sc * 512:(sc + 1) * 512],
                                    start=(et == 0),
                                    stop=(et == ET - 1),
                                    skip_group_check=True,
                                )
                        nc.scalar.activation(
                            qk_sb[ft],
                            ps,
                            mybir.ActivationFunctionType.Identity,
                            bias=bqk_sb[:, ft:ft + 1],
                        )
                with tc.tile_pool(name="ph1psv", bufs=4, space="PSUM") as ppv:
                    for st in range(ST):
                        nc.vector.memset(v_sb[st], 1.0)
                        ps = ppv.tile([128, 512], F32, tag="p1v")
                        for et in range(ET):
                            nc.tensor.matmul(
                                ps,
                                xT_sb[et][:, st * 128:(st + 1) * 128],
                                wv_sb[et],
                                start=(et == 0),
                                stop=False,
                            )
                        nc.tensor.matmul(
                            ps, ones, bv_sb, start=False, stop=True,
                        )
                        nc.scalar.activation(
                            v_sb[st].rearrange("p (h d) -> p h d", d=D + 1)[:, :, 0:D],
                            ps.rearrange("p (h d) -> p h d", d=D),
                            mybir.ActivationFunctionType.Copy,
                        )

            # ---- phase 2: attention, software-pipelined across (h, ih) ----
            # While computing scores/exp/AV for pass n, interleave the PE
            # transposes + normalize-copies + stores of pass n-1 so the PE
            # stream never waits on ACT (keeps the HAM clock at 2.4 GHz).
            with (
                tc.tile_pool(name="expp", bufs=2 * JT + 4) as expp,
                tc.tile_pool(name="stagep", bufs=3) as stagep,
                tc.tile_pool(name="outp", bufs=2) as outp,
                tc.tile_pool(name="smallp", bufs=2) as smallp,
                tc.tile_pool(name="dramp", bufs=2, space="DRAM") as dramp,
                tc.tile_pool(name="psS", bufs=2, space="PSUM") as psS,
                tc.tile_pool(name="psT", bufs=2, space="PSUM") as psT,
                tc.tile_pool(name="psC", bufs=1, space="PSUM") as psC,
            ):
                passes = [(h, ih) for h in range(NH) for ih in range(IH)]

                class Pass:
                    pass

                def start_pass(h, ih):
                    p = Pass()
                    p.h, p.ih = h, ih
                    p.i0 = ih * IW
                    p.ctx_ps = psC.tile([D + 1, IW], F32, tag="ctx", name="ctx_ps")
                    p.exp_tiles = []
                    p.stage = None
                    p.ps_sc = []
                    p.copy_clock = 0
                    return p

                def scores_exp(p, jt):
                    qt = qk_sb[p.h // 2]
                    kt = qk_sb[NH // 2 + p.h // 2]
                    po = (p.h % 2) * D
                    etile = expp.tile([128, IW], BF16, tag="exp", name="etile")
                    ps = psS.tile([128, IW], F32, tag="sc", name="sc_ps")
                    for ic in range(ICN):
                        nc.tensor.matmul(
                            ps[:, ic * 512:(ic + 1) * 512],
                            kt[po:po + D, jt * 128:(jt + 1) * 128],
                            qt[po:po + D, p.i0 + ic * 512: p.i0 + (ic + 1) * 512],
                            start=True,
                            stop=True,
                            skip_group_check=True,
                        )
                    nc.scalar.activation(
                        etile,
                        ps,
                        mybir.ActivationFunctionType.Exp,
                        scale=float(SCALE),
                    )
                    p.exp_tiles.append(etile)

                def av(p, jt):
                    etile = p.exp_tiles[jt]
                    for ic in range(ICN):
                        nc.tensor.matmul(
                            p.ctx_ps[:, ic * 512:(ic + 1) * 512],
                            v_sb[jt][:, p.h * (D + 1): p.h * (D + 1) + D + 1],
                            etile[:, ic * 512:(ic + 1) * 512],
                            start=(jt == 0),
                            stop=(jt == JT - 1),
                            skip_group_check=True,
                        )

                def finish_accum(p):
                    """After the last AV: reciprocals, free ctx psum, set up
                    the normalize inputs for the transpose pass."""
                    recip_row = smallp.tile([1, IW], F32, tag="recip", name="recip_row")
                    nc.vector.reciprocal(recip_row, p.ctx_ps[D:D + 1, :])
                    cu = smallp.tile([D, IW], BF16, tag="cu", name="ctx_unnorm")
                    nc.scalar.copy(cu, p.ctx_ps[0:D, :])
                    p.ctx_unnorm = cu
                    # transposed reciprocals for the per-partition stage scale
                    rt_ps = psT.tile([128, 512], F32, tag="tp", name="rt_ps")
                    for c in range(IW // 128):
                        nc.tensor.transpose(
                            rt_ps[:, c:c + 1],
                            recip_row[:, c * 128:(c + 1) * 128],
                            idf,
                        )
                    rt_sb = smallp.tile([128, IW // 128], F32, tag="rt", name="rt_sb")
                    nc.vector.tensor_copy(rt_sb, rt_ps[:, 0:IW // 128])
                    p.rt_sb = rt_sb
                    # broadcast reciprocal row via DRAM bounce (bf16)
                    rscratch = dramp.tile([1, IW], BF16, tag="rscratch", name="rscratch")
                    nc.gpsimd.dma_start(out=rscratch, in_=recip_row)
                    rb = smallp.tile([D, IW], BF16, tag="rb", name="rb")
                    bcast_ap = bass.AP(
                        tensor=rscratch.tensor,
                        offset=rscratch.offset,
                        ap=[[0, D]] + [list(q) for q in rscratch.ap[1:]],
                    )
                    nc.gpsimd.dma_start(out=rb, in_=bcast_ap)
                    p.rb = rb

                def norm_ctx(p):
                    po = (p.h % 2) * D
                    nc.vector.tensor_mul(
                        ctx_sb[p.h // 2][po:po + D, p.i0:p.i0 + IW],
                        p.ctx_unnorm,
                        p.rb,
                    )

                def transpose_group(p, c):
                    """Transpose group c (0..15) of pass p: 8 j-tiles of one
                    128-wide i-tile -> one stage copy; store when row done."""
                    it = c // 2
                    g = c % 2
                    if g == 0:
                        p.stage = stagep.tile([128, S], BF16, tag="stage", name="stage")
                    tp = psT.tile([128, IW], BF16, tag="tp", name="tp_ps")
                    for k in range(8):
                        jt = g * 8 + k
                        nc.tensor.transpose(
                            tp[:, k * 128:(k + 1) * 128],
                            p.exp_tiles[jt][:, it * 128:(it + 1) * 128],
                            ident,
                        )
                    dst = p.stage[:, g * IW:(g + 1) * IW]
                    scal = p.rt_sb[:, it:it + 1]
                    nc.vector.tensor_scalar_mul(dst, tp, scal)
                    if g == 1:
                        nc.sync.dma_start(
                            out=attn_o[p.h, p.i0 + it * 128: p.i0 + (it + 1) * 128, :],
                            in_=p.stage,
                        )

                prev = None
                for idx in range(len(passes) + 1):
                    cur = start_pass(*passes[idx]) if idx < len(passes) else None
                    for c in range(JT):
                        if cur is not None:
                            scores_exp(cur, c)
                            if c >= 2:
                                av(cur, c - 2)
                        if prev is not None:
                            transpose_group(prev, c)
                            if c == 2:
                                norm_ctx(prev)
                    if cur is not None:
                        av(cur, JT - 2)
                        av(cur, JT - 1)
                        finish_accum(cur)
                    prev = cur

                # ---- phase 3: projection partial ----
                for sti in range(ST):
                    osb = outp.tile([128, E], F32, tag="out", name="osb")
                    for ec in range(E // 512):
                        ps = psS.tile([128, 512], F32, tag="sc", name="proj_ps")
                        for hp in range(NH // 2):
                            nc.tensor.matmul(
                                ps,
                                ctx_sb[hp][:, sti * 128:(sti + 1) * 128],
                                wproj_sb[hp][:, ec * 512:(ec + 1) * 512],
                                start=(hp == 0),
                                stop=(hp == NH // 2 - 1),
                            )
                        nc.scalar.copy(osb[:, ec * 512:(ec + 1) * 512], ps)
                    nc.sync.dma_start(out=out_o[sti * 128:(sti + 1) * 128, :], in_=osb)

    nc.finalize()
    return nc


_NC = None
LAST_EXEC_NS = None
LAST_RESULTS = None


def kernel(x, W_qkv, b_qkv, W_proj, b_proj):
    global _NC, LAST_EXEC_NS, LAST_RESULTS
    x = np.asarray(x, dtype=np.float32)
    W_qkv = np.asarray(W_qkv, dtype=np.float32)
    b_qkv = np.asarray(b_qkv, dtype=np.float32)
    W_proj = np.asarray(W_proj, dtype=np.float32)
    b_proj = np.asarray(b_proj, dtype=np.float32)

    if _NC is None:
        _NC = _build_kernel()
    nc = _NC

    # per-head column blocks of W_qkv: q at h*D, k at E + h*D, v at 2E + h*D
    Wq = W_qkv[:, :E].reshape(E, H, D)
    Wk = W_qkv[:, E:2 * E].reshape(E, H, D)
    Wv = W_qkv[:, 2 * E:].reshape(E, H, D)
    bq = b_qkv[:E].reshape(H, D)
    bk = b_qkv[E:2 * E].reshape(H, D)
    bvv = b_qkv[2 * E:].reshape(H, D)

    in_maps = []
    for c in range(NCORES):
        b = c // 2
        hh = c % 2
        hs = slice(hh * NH, (hh + 1) * NH)
        wqk_l = np.concatenate(
            [Wq[:, hs].reshape(E, HD), Wk[:, hs].reshape(E, HD)], axis=1
        )
        bqk_l = np.concatenate([bq[hs].reshape(HD), bk[hs].reshape(HD)])
        wv_l = Wv[:, hs].reshape(E, HD)
        bv_l = bvv[hs].reshape(HD)
        wproj_l = W_proj[hh * HD:(hh + 1) * HD, :]
        in_maps.append({
            "xT": np.ascontiguousarray(x[b].T).astype(NP_BF16),
            "wqk": wqk_l.astype(NP_BF16),
            "wv": wv_l.astype(NP_BF16),
            "bqk": bqk_l.astype(np.float32),
            "bv": bv_l.astype(NP_BF16),
            "wproj": wproj_l.astype(NP_BF16),
        })

    trace = bool(os.environ.get("TRNKERNEL_TRACE"))
    kwargs = {}
    if trace:
        _install_ntff_hook()
        kwargs["trace"] = True
        tdir = os.environ.get("TRNKERNEL_TRACE_DIR")
        if tdir:
            os.makedirs(tdir, exist_ok=True)
            kwargs["tmpdir"] = tdir
    res = run_bass_kernel_spmd(nc, in_maps, core_ids=list(range(NCORES)), **kwargs)
    LAST_EXEC_NS = res.exec_time_ns
    LAST_RESULTS = res

    attn = np.empty((B, H, S, S), np.float32)
    out = np.empty((B, S, E), np.float32)
    for c in range(NCORES):
        b = c // 2
        hh = c % 2
        attn[b, hh * NH:(hh + 1) * NH] = res.results[c]["attn_o"].astype(np.float32)
    for b in range(B):
        out[b] = res.results[2 * b]["out_o"] + res.results[2 * b + 1]["out_o"] + b_proj
    return out, attn


# revision 9
# speedup vs baseline: 1.2148x; 1.0073x over previous
"""Multi-head attention (B=4, S=2048, E=1024, H=16) on 8 Trainium2 NeuronCores.

Sharding: 2 cores per batch element (data-parallel over B=4), each core
computes 8 of the 16 heads (tensor-parallel over H). Each core:
  - qkT = (W_qk_local)^T @ x^T            [1024f, 2048s]  (f = q|k heads)
  - v   = x @ W_v_local (+bias via ones)  [2048s, 8*65]   (65th col = ones)
  - per head: scoresT = kT^T q (j on partitions), exp on ACT (no max
    subtraction needed -- scores are small), AV via v_aug^T @ expT which
    also yields the softmax denominators in the extra row, transposed
    normalized attention written via PE transposes, ctx normalized with a
    broadcast reciprocal, projection partial out = ctxT^T @ W_proj_local.
Host sums the two per-batch projection partials and adds b_proj.
"""

import os
import sys
import types

for _p in ("/opt/trn_rl_repo", "/root/.axon_site/_ro/trn_rl_repo"):
    if os.path.isdir(_p) and _p not in sys.path:
        sys.path.append(_p)

import numpy as np
import ml_dtypes

import concourse.bass as bass
import concourse.bacc as bacc
import concourse.tile as tile
from concourse import mybir
from concourse.bass_utils import run_bass_kernel_spmd
from concourse.masks import make_identity
from concourse.vector_clock import ScopedClock, VectorClock

BF16 = mybir.dt.bfloat16
F32 = mybir.dt.float32
NP_BF16 = ml_dtypes.bfloat16

B, S, E, H, D = 4, 2048, 1024, 16, 64
NCORES = 8
NH = 8          # local heads per core
HD = NH * D     # 512 local head-dim columns per of q/k/v
SCALE = 1.0 / np.sqrt(D)


def _install_ntff_hook():
    """The agent image's antenv lacks axon_hooks; inject it so trace=True works."""
    try:
        import antenv.axon_hooks  # noqa: F401
        return
    except ImportError:
        pass
    try:
        import antenv
        from trn_agent_boot.trn_boot import _ntff_profile_via_ctypes
    except ImportError:
        return
    mod = types.ModuleType("antenv.axon_hooks")
    mod._hook = None

    def set_axon_ntff_profile_hook(h):
        mod._hook = h

    def get_axon_ntff_profile_hook():
        return mod._hook

    mod.set_axon_ntff_profile_hook = set_axon_ntff_profile_hook
    mod.get_axon_ntff_profile_hook = get_axon_ntff_profile_hook
    sys.modules["antenv.axon_hooks"] = mod
    antenv.axon_hooks = mod
    try:
        mod._hook = _ntff_profile_via_ctypes("/opt/axon/libaxon_pjrt.so")
    except Exception:
        mod._hook = None


class ChunkedDrainTileContext(tile.TileContext):
    """TileContext whose exit drain splits semaphore waits across multiple
    drain instructions (this walrus encodes only one sync wait per CTRL)."""

    def _drain_and_barrier(self, tick_clock, wait_clock):
        gc = tick_clock.global_clock
        n = len(gc)
        ticks = [(p, gc[p]) for p in range(n) if gc[p] > 0]
        if not ticks:
            self.nc.sync.drain()
        for p, t in ticks:
            vc = VectorClock([0] * n)
            vc.require_at_least(p, t)
            d = self.nc.sync.drain()
            wait_clock.add_sem_waits(d.ins, ScopedClock({None: vc}))
        self.nc.all_engine_barrier()
        assert self.sems is not None
        popped = self.nc._tile_sem_poison_stack.pop()
        assert popped is self._sem_poison
        self.nc.clear_and_free_semaphores(list(self.sems.allocated().values()))
        self.nc.all_engine_barrier()


def _build_kernel():
    nc = bacc.Bacc("TRN2")

    xT = nc.dram_tensor("xT", [E, S], BF16, kind="ExternalInput")
    wqk = nc.dram_tensor("wqk", [E, 2 * HD], BF16, kind="ExternalInput")
    wv = nc.dram_tensor("wv", [E, HD], BF16, kind="ExternalInput")
    bqk = nc.dram_tensor("bqk", [2 * HD], F32, kind="ExternalInput")
    bv = nc.dram_tensor("bv", [HD], BF16, kind="ExternalInput")
    wproj = nc.dram_tensor("wproj", [HD, E], BF16, kind="ExternalInput")
    attn_o = nc.dram_tensor("attn_o", [NH, S, S], BF16, kind="ExternalOutput")
    out_o = nc.dram_tensor("out_o", [S, E], F32, kind="ExternalOutput")

    ET = E // 128   # 8 e-tiles (contraction tiles for qkv matmuls)
    ST = S // 128   # 16 s-tiles
    JT = S // 128   # 16 j-tiles per head
    IH = 2          # i halves
    IW = S // IH    # 1024 i per half
    ICN = IW // 512  # 2 512-wide i chunks per half

    with ChunkedDrainTileContext(nc) as tc:
        with tc.tile_pool(name="persist", bufs=1) as persist:
            ident = persist.tile([128, 128], BF16)
            make_identity(nc, ident)
            idf = persist.tile([1, 1], F32)
            nc.vector.memset(idf, 1.0)
            ones = persist.tile([1, 128], BF16)
            nc.vector.memset(ones, 1.0)
            bqk_sb = persist.tile([128, 2 * HD // 128], F32)
            nc.gpsimd.dma_start(out=bqk_sb, in_=bqk.rearrange("(t p) -> p t", p=128))
            bv_sb = persist.tile([1, HD], BF16)
            nc.gpsimd.dma_start(out=bv_sb, in_=bv[None, :])

            # persistent activations
            qk_sb = [persist.tile([128, S], BF16, tag=f"qk{i}", name=f"qk{i}")
                     for i in range(2 * HD // 128)]
            v_sb = [persist.tile([128, NH * (D + 1)], BF16, tag=f"v{i}", name=f"v{i}")
                    for i in range(ST)]
            wproj_sb = [persist.tile([128, E], BF16, tag=f"wp{i}", name=f"wp{i}")
                        for i in range(HD // 128)]
            ctx_sb = [persist.tile([128, S], BF16, tag=f"ctx{i}", name=f"ctx{i}")
                      for i in range(NH // 2)]

            for t in range(HD // 128):
                nc.sync.dma_start(out=wproj_sb[t], in_=wproj[t * 128:(t + 1) * 128, :])

            # ---- phase 1: qkT and v ----
            with tc.tile_pool(name="ph1", bufs=1) as ph1:
                xT_sb = [ph1.tile([128, S], BF16, tag=f"xT{i}", name=f"xT{i}") for i in range(ET)]
                wqk_sb = [ph1.tile([128, 2 * HD], BF16, tag=f"wqk{i}", name=f"wqk{i}")
                          for i in range(ET)]
                wv_sb = [ph1.tile([128, HD], BF16, tag=f"wv{i}", name=f"wv{i}") for i in range(ET)]
                for t in range(ET):
                    nc.sync.dma_start(out=xT_sb[t], in_=xT[t * 128:(t + 1) * 128, :])
                    nc.sync.dma_start(out=wqk_sb[t], in_=wqk[t * 128:(t + 1) * 128, :])
                    nc.sync.dma_start(out=wv_sb[t], in_=wv[t * 128:(t + 1) * 128, :])

                with tc.tile_pool(name="ph1ps", bufs=4, space="PSUM") as pp:
                    for ft in range(2 * HD // 128):
                        for half in range(2):
                            ps = pp.tile([128, S // 2], F32, tag="p1w", name="p1w")
                            for sc in range(2):
                                scg = half * 2 + sc
                                for et in range(ET):
                                    nc.tensor.matmul(
                                        ps[:, sc * 512:(sc + 1) * 512],
                                        wqk_sb[et][:, ft * 128:(ft + 1) * 128],
                                        xT_sb[et][:, scg * 512:(scg + 1) * 512],
                                        start=(et == 0),
                                        stop=(et == ET - 1),
                                        skip_group_check=True,
                                    )
                            nc.scalar.activation(
                                qk_sb[ft][:, half * (S // 2):(half + 1) * (S // 2)],
                                ps,
                                mybir.ActivationFunctionType.Identity,
                                bias=bqk_sb[:, ft:ft + 1],
                            )
                with tc.tile_pool(name="ph1psv", bufs=4, space="PSUM") as ppv:
                    for st in range(ST):
                        nc.vector.memset(v_sb[st], 1.0)
                        ps = ppv.tile([128, 512], F32, tag="p1v")
                        for et in range(ET):
                            nc.tensor.matmul(
                                ps,
                                xT_sb[et][:, st * 128:(st + 1) * 128],
                                wv_sb[et],
                                start=(et == 0),
                                stop=False,
                            )
                        nc.tensor.matmul(
                            ps, ones, bv_sb, start=False, stop=True,
                        )
                        nc.scalar.activation(
                            v_sb[st].rearrange("p (h d) -> p h d", d=D + 1)[:, :, 0:D],
                            ps.rearrange("p (h d) -> p h d", d=D),
                            mybir.ActivationFunctionType.Copy,
                        )

            # ---- phase 2: attention, software-pipelined across (h, ih) ----
            # While computing scores/exp/AV for pass n, interleave the PE
            # transposes + normalize-copies + stores of pass n-1 so the PE
            # stream never waits on ACT (keeps the HAM clock at 2.4 GHz).
            with (
                tc.tile_pool(name="expp", bufs=2 * JT + 4) as expp,
                tc.tile_pool(name="stagep", bufs=3) as stagep,
                tc.tile_pool(name="outp", bufs=2) as outp,
                tc.tile_pool(name="smallp", bufs=2) as smallp,
                tc.tile_pool(name="dramp", bufs=2, space="DRAM") as dramp,
                tc.tile_pool(name="psS", bufs=2, space="PSUM") as psS,
                tc.tile_pool(name="psT", bufs=2, space="PSUM") as psT,
                tc.tile_pool(name="psC", bufs=1, space="PSUM") as psC,
            ):
                passes = [(h, ih) for h in range(NH) for ih in range(IH)]

                class Pass:
                    pass

                def start_pass(h, ih):
                    p = Pass()
                    p.h, p.ih = h, ih
                    p.i0 = ih * IW
                    p.ctx_ps = psC.tile([D + 1, IW], F32, tag="ctx", name="ctx_ps")
                    p.exp_tiles = []
                    p.stage = None
                    p.ps_sc = []
                    p.copy_clock = 0
                    return p

                def scores_exp(p, jt):
                    qt = qk_sb[p.h // 2]
                    kt = qk_sb[NH // 2 + p.h // 2]
                    po = (p.h % 2) * D
                    etile = expp.tile([128, IW], BF16, tag="exp", name="etile")
                    ps = psS.tile([128, IW], F32, tag="sc", name="sc_ps")
                    for ic in range(ICN):
                        nc.tensor.matmul(
                            ps[:, ic * 512:(ic + 1) * 512],
                            kt[po:po + D, jt * 128:(jt + 1) * 128],
                            qt[po:po + D, p.i0 + ic * 512: p.i0 + (ic + 1) * 512],
                            start=True,
                            stop=True,
                            skip_group_check=True,
                        )
                    nc.scalar.activation(
                        etile,
                        ps,
                        mybir.ActivationFunctionType.Exp,
                        scale=float(SCALE),
                    )
                    p.exp_tiles.append(etile)

                def av(p, jt):
                    etile = p.exp_tiles[jt]
                    for ic in range(ICN):
                        nc.tensor.matmul(
                            p.ctx_ps[:, ic * 512:(ic + 1) * 512],
                            v_sb[jt][:, p.h * (D + 1): p.h * (D + 1) + D + 1],
                            etile[:, ic * 512:(ic + 1) * 512],
                            start=(jt == 0),
                            stop=(jt == JT - 1),
                            skip_group_check=True,
                        )

                def finish_accum(p):
                    """After the last AV: copy sums + unnormalized ctx out of
                    PSUM (frees the bank), bounce sums through DRAM to get
                    both the transposed [128, 8] view and the broadcast
                    [64, IW] view, then reciprocal both on DVE. No PE ops —
                    keeps the PE stream dense."""
                    sums_row = smallp.tile([1, IW], F32, tag="sums", name="sums_row")
                    nc.scalar.copy(sums_row, p.ctx_ps[D:D + 1, :])
                    cu = smallp.tile([D, IW], BF16, tag="cu", name="ctx_unnorm")
                    nc.scalar.copy(cu, p.ctx_ps[0:D, :])
                    p.ctx_unnorm = cu
                    rscratch = dramp.tile([1, IW], F32, tag="rscratch", name="rscratch")
                    nc.gpsimd.dma_start(out=rscratch, in_=sums_row)
                    # transposed view: [128, 8] with (p, c) = sums[c*128 + p]
                    sumsT = smallp.tile([128, IW // 128], F32, tag="rt0", name="sumsT")
                    t_ap = bass.AP(
                        tensor=rscratch.tensor,
                        offset=rscratch.offset,
                        ap=[[1, 128], [128, IW // 128]],
                    )
                    nc.gpsimd.dma_start(out=sumsT, in_=t_ap)
                    rt_sb = smallp.tile([128, IW // 128], F32, tag="rt", name="rt_sb")
                    nc.vector.reciprocal(rt_sb, sumsT)
                    p.rt_sb = rt_sb
                    # broadcast view: [D, IW] (partition-stride 0)
                    rbs = smallp.tile([D, IW], F32, tag="rbs", name="rbs")
                    b_ap = bass.AP(
                        tensor=rscratch.tensor,
                        offset=rscratch.offset,
                        ap=[[0, D]] + [list(q) for q in rscratch.ap[1:]],
                    )
                    nc.gpsimd.dma_start(out=rbs, in_=b_ap)
                    nc.vector.reciprocal(rbs, rbs)
                    p.rb = rbs

                def norm_ctx(p):
                    po = (p.h % 2) * D
                    nc.vector.tensor_mul(
                        ctx_sb[p.h // 2][po:po + D, p.i0:p.i0 + IW],
                        p.ctx_unnorm,
                        p.rb,
                    )

                def transpose_group(p, c):
                    """Transpose group c (0..15) of pass p: 8 j-tiles of one
                    128-wide i-tile -> one stage copy; store when row done."""
                    it = c // 2
                    g = c % 2
                    if g == 0:
                        p.stage = stagep.tile([128, S], BF16, tag="stage", name="stage")
                    tp = psT.tile([128, IW], BF16, tag="tp", name="tp_ps")
                    for k in range(8):
                        jt = g * 8 + k
                        nc.tensor.transpose(
                            tp[:, k * 128:(k + 1) * 128],
                            p.exp_tiles[jt][:, it * 128:(it + 1) * 128],
                            ident,
                        )
                    dst = p.stage[:, g * IW:(g + 1) * IW]
                    scal = p.rt_sb[:, it:it + 1]
                    nc.vector.tensor_scalar_mul(dst, tp, scal)
                    if g == 1:
                        nc.sync.dma_start(
                            out=attn_o[p.h, p.i0 + it * 128: p.i0 + (it + 1) * 128, :],
                            in_=p.stage,
                        )

                prev = None
                for idx in range(len(passes) + 1):
                    cur = start_pass(*passes[idx]) if idx < len(passes) else None
                    for c in range(JT):
                        if cur is not None:
                            scores_exp(cur, c)
                            if c >= 2:
                                av(cur, c - 2)
                        if prev is not None:
                            transpose_group(prev, c)
                            if c == 2:
                                norm_ctx(prev)
                    if cur is not None:
                        av(cur, JT - 2)
                        av(cur, JT - 1)
                        finish_accum(cur)
                    prev = cur

                # ---- phase 3: projection partial ----
                for sti in range(ST):
                    osb = outp.tile([128, E], F32, tag="out", name="osb")
                    for ec in range(E // 512):
                        ps = psS.tile([128, 512], F32, tag="sc", name="proj_ps")
                        for hp in range(NH // 2):
                            nc.tensor.matmul(
                                ps,
                                ctx_sb[hp][:, sti * 128:(sti + 1) * 128],
                                wproj_sb[hp][:, ec * 512:(ec + 1) * 512],
                                start=(hp == 0),
                                stop=(hp == NH // 2 - 1),
                            )
                        nc.scalar.copy(osb[:, ec * 512:(ec + 1) * 512], ps)
                    nc.sync.dma_start(out=out_o[sti * 128:(sti + 1) * 128, :], in_=osb)

    nc.finalize()
    return nc


_NC = None
LAST_EXEC_NS = None
LAST_RESULTS = None


def kernel(x, W_qkv, b_qkv, W_proj, b_proj):
    global _NC, LAST_EXEC_NS, LAST_RESULTS
    x = np.asarray(x, dtype=np.float32)
    W_qkv = np.asarray(W_qkv, dtype=np.float32)
    b_qkv = np.asarray(b_qkv, dtype=np.float32)
    W_proj = np.asarray(W_proj, dtype=np.float32)
    b_proj = np.asarray(b_proj, dtype=np.float32)

    if _NC is None:
        _NC = _build_kernel()
    nc = _NC

    # per-head column blocks of W_qkv: q at h*D, k at E + h*D, v at 2E + h*D
    Wq = W_qkv[:, :E].reshape(E, H, D)
    Wk = W_qkv[:, E:2 * E].reshape(E, H, D)
    Wv = W_qkv[:, 2 * E:].reshape(E, H, D)
    bq = b_qkv[:E].reshape(H, D)
    bk = b_qkv[E:2 * E].reshape(H, D)
    bvv = b_qkv[2 * E:].reshape(H, D)

    in_maps = []
    for c in range(NCORES):
        b = c // 2
        hh = c % 2
        hs = slice(hh * NH, (hh + 1) * NH)
        wqk_l = np.concatenate(
            [Wq[:, hs].reshape(E, HD), Wk[:, hs].reshape(E, HD)], axis=1
        )
        bqk_l = np.concatenate([bq[hs].reshape(HD), bk[hs].reshape(HD)])
        wv_l = Wv[:, hs].reshape(E, HD)
        bv_l = bvv[hs].reshape(HD)
        wproj_l = W_proj[hh * HD:(hh + 1) * HD, :]
        in_maps.append({
            "xT": np.ascontiguousarray(x[b].T).astype(NP_BF16),
            "wqk": wqk_l.astype(NP_BF16),
            "wv": wv_l.astype(NP_BF16),
            "bqk": bqk_l.astype(np.float32),
            "bv": bv_l.astype(NP_BF16),
            "wproj": wproj_l.astype(NP_BF16),
        })

    trace = bool(os.environ.get("TRNKERNEL_TRACE"))
    kwargs = {}
    if trace:
        _install_ntff_hook()
        kwargs["trace"] = True
        tdir = os.environ.get("TRNKERNEL_TRACE_DIR")
        if tdir:
            os.makedirs(tdir, exist_ok=True)
            kwargs["tmpdir"] = tdir
    res = run_bass_kernel_spmd(nc, in_maps, core_ids=list(range(NCORES)), **kwargs)
    LAST_EXEC_NS = res.exec_time_ns
    LAST_RESULTS = res

    attn = np.empty((B, H, S, S), np.float32)
    out = np.empty((B, S, E), np.float32)
    for c in range(NCORES):
        b = c // 2
        hh = c % 2
        attn[b, hh * NH:(hh + 1) * NH] = res.results[c]["attn_o"].astype(np.float32)
    for b in range(B):
        out[b] = res.results[2 * b]["out_o"] + res.results[2 * b + 1]["out_o"] + b_proj
    return out, attn
